# revision 1
# baseline (speedup 1.0000x reference)
"""Trainium2 Bass kernel for the nn_CA depthwise-conv CA step (v2, bf16).

Pipeline per image: depthwise 5x5 conv (D4-symmetrized, zero-mean kernel,
SAME padding) + leaky_relu; 1x1 conv (8x8 channel GEMM) + bias + leaky_relu;
1x1 conv + bias + psi residual + tanh.

Strategy: pure data parallel over 8 NeuronCores (256 images each), all
tensors bf16 (numpy sim: rel err 7.5e-3 << 2e-2 tol), fp32 PSUM accumulate.

Two on-chip layouts per core:
 - "pair" (stage 1): partition p = b*64 + j*4 + G for channel c=2q+b at
   y = 16G + j; free = (q, image, padded x).  The y-conv is a (permuted)
   banded Toeplitz lhsT; D4 x-symmetry folds the 5 x-taps into 3 matmul
   terms: center column + s1 = psi(x-1)+psi(x+1) + s2 = psi(x-2)+psi(x+2),
   with s1/s2 computed on the DVE.  3 matmuls per channel pair instead of
   the baseline's 4-5.
 - "shuf" (stages 2+3): partition p = c*16 + j (all 8 channels in the
   partition dim); free = (G, image, x).  The 8x8 channel GEMMs become ONE
   matmul per tile (full 8-channel contraction via w.T (x) I16 lhsT) -- 4x
   fewer PE passes than the baseline's I64-block formulation.  The psi
   residual rides an accumulating I128 matmul from a host-packed shuf copy
   of psi.

The pair->shuf z1 reshuffle is a partition permutation done via a DRAM
round-trip (2 full-128-partition DMAs per supergroup through a 2-slot DRAM
scratch ring; the (q,b,j,G,m) scratch layout makes both directions single
DMAs with >=2KB runs).  Measured much faster than SBUF->SBUF shuffle DMAs,
which cost ~1.6us each due to partition-concentrated writes.  Biases + leaky_relu/tanh are fused into ScalarE activations reading
PSUM directly.  bias1 is always zero per the problem spec and is not
applied (same as the baseline).
"""

import numpy as np
import ml_dtypes

BF16 = ml_dtypes.bfloat16

BS, H, RES = 2048, 8, 64
NCORES = 8
IPC = BS // NCORES   # images per core = 256
SG = 16              # images per supergroup (DMA granularity)
NSG = IPC // SG      # 16 supergroups
S8 = 8               # images per PSUM step
NST = SG // S8       # 2 steps per supergroup
XP = RES + 4         # x-padded width

# free-dim sizes
PTW = 4 * SG * XP        # psi pair tile width  = 4352
SW = 4 * SG * RES        # s1/s2 tile width     = 4096
ZW = 4 * SG * RES        # z1 pair/shuf width   = 4096
OW = 4 * SG * RES        # out tile width       = 4096

_CACHE = {}

# partition interleave: y = 16*G + j  ->  pi(y) = j*4 + G
_PI = np.array([(y % 16) * 4 + y // 16 for y in range(64)])


def _totalistic(x):
    z = 0.125 * (x + x[:, :, ::-1, :] + x[:, :, :, ::-1] + x[:, :, ::-1, ::-1])
    xt = np.swapaxes(x, 2, 3)
    z = z + 0.125 * (xt + xt[:, :, ::-1, :] + xt[:, :, :, ::-1] + xt[:, :, ::-1, ::-1])
    return z - z.mean(axis=(2, 3), keepdims=True)


def _host_pack_weights(filter1, w2, w3):
    """-> wm [15, 128, 128] bf16: 12 stage-1 permuted-Toeplitz lhsTs
    (t*4+q), W2 shuf lhsT (12), W3 shuf lhsT (13), I128 (14)."""
    K = _totalistic(filter1.astype(np.float32))[:, 0]  # [8,5,5]
    wm = np.zeros((15, 128, 128), np.float32)
    # stage 1: z[y'] = sum_dy K[dy+2, col_t] * s_t[y'+dy]
    for t, col in enumerate((2, 1, 0)):
        for q in range(4):
            m = wm[t * 4 + q]
            for b in range(2):
                k = K[2 * q + b, :, col]
                for yp in range(64):
                    for dy in range(-2, 3):
                        y = yp + dy
                        if 0 <= y < 64:
                            m[b * 64 + _PI[y], b * 64 + _PI[yp]] = k[dy + 2]
    wm[12] = np.kron(w2.T.astype(np.float32), np.eye(16, dtype=np.float32))
    wm[13] = np.kron(w3.T.astype(np.float32), np.eye(16, dtype=np.float32))
    wm[14] = np.eye(128, dtype=np.float32)
    return wm.astype(BF16)


def _host_pack_biases(b2, b3):
    bv = np.zeros((128, 2), np.float32)
    p = np.arange(128)
    bv[:, 0] = b2[p // 16]
    bv[:, 1] = b3[p // 16]
    return bv


def _pack_psi_pair(psi):
    """[BS,H,RES,RES] -> [NC, 128, NSG, PTW] bf16, partition b*64+j*4+G,
    free (q, m, x+2) with zero x-pad baked in."""
    # (core, sg, m, q, b, G, j, x)
    v = psi.reshape(NCORES, NSG, SG, 4, 2, 4, 16, RES)
    v = v.transpose(0, 4, 6, 5, 1, 3, 2, 7)  # core, b, j, G, sg, q, m, x
    vp = np.zeros((NCORES, 2, 16, 4, NSG, 4, SG, XP), np.float32)
    vp[..., 2:2 + RES] = v
    return np.ascontiguousarray(vp).astype(BF16).reshape(NCORES, 128, NSG, PTW)


def _pack_psi_shuf(psi):
    """[BS,H,RES,RES] -> [NC, 128, NSG, SW] bf16, partition c*16+j,
    free (G, m, x)."""
    # (core, sg, m, c, G, j, x)
    v = psi.reshape(NCORES, NSG, SG, H, 4, 16, RES)
    v = v.transpose(0, 3, 5, 1, 4, 2, 6)  # core, c, j, sg, G, m, x
    return np.ascontiguousarray(v).astype(BF16).reshape(NCORES, 128, NSG, SW)


def _unpack_out(parts):
    """list of [128, NSG, OW] bf16 per core -> [BS,H,RES,RES] fp32"""
    v = np.stack(parts).astype(np.float32)  # [NC, 128, NSG, OW]
    v = v.reshape(NCORES, H, 16, NSG, 4, SG, RES)  # core, c, j, sg, G, m, x
    v = v.transpose(0, 3, 5, 1, 4, 2, 6)  # core, sg, m, c, G, j, x
    return np.ascontiguousarray(v).reshape(BS, H, RES, RES)


def _build_program(reps=1, nsg_io=NSG, probe=None, swdge_shuffle=False):
    """nsg_io < NSG builds a timing variant: identical instruction stream,
    but DRAM I/O cycles through a small nsg_io-supergroup buffer so the
    host<->device shipping (which axon does per exec) is negligible."""
    import concourse.bacc as bacc
    import concourse.tile as tile
    from concourse import mybir

    dt = mybir.dt
    nc = bacc.Bacc("TRN2", target_bir_lowering=False, debug=False,
                   num_devices=NCORES)
    psi = nc.dram_tensor("psi", [128, nsg_io, PTW], dt.bfloat16,
                         kind="ExternalInput").ap()
    psis = nc.dram_tensor("psis", [128, nsg_io, SW], dt.bfloat16,
                          kind="ExternalInput").ap()
    wm = nc.dram_tensor("wm", [15, 128, 128], dt.bfloat16,
                        kind="ExternalInput").ap()
    bv = nc.dram_tensor("bv", [128, 2], dt.float32,
                        kind="ExternalInput").ap()
    out = nc.dram_tensor("out", [128, nsg_io, OW], dt.bfloat16,
                         kind="ExternalOutput").ap()
    # DRAM scratch for the z1 pair->shuf permutation (2-slot ring).
    # dims: (slot, q, b, j, G, m*x) -- chosen so both directions are single
    # full-128-partition DMAs: write groups (b j G), read groups (q b j).
    zs = nc.dram_tensor("zs", [4, 4, 2, 16, 4, SG * RES], dt.bfloat16,
                        kind="Internal").ap()

    LR = mybir.ActivationFunctionType.Lrelu
    TH = mybir.ActivationFunctionType.Tanh

    with tile.TileContext(nc) as tc:
        from contextlib import ExitStack

        with ExitStack() as ctx:
            const = ctx.enter_context(tc.tile_pool(name="const", bufs=1))
            psip = ctx.enter_context(tc.tile_pool(name="psip", bufs=3))
            spool = ctx.enter_context(tc.tile_pool(name="spool", bufs=2))
            zpool = ctx.enter_context(tc.tile_pool(name="zpool", bufs=3))
            opool = ctx.enter_context(tc.tile_pool(name="opool", bufs=3))
            psum = ctx.enter_context(tc.tile_pool(name="psum", bufs=2,
                                                  space="PSUM"))

            wt = const.tile([128, 15 * 128], dt.bfloat16)
            nc.sync.dma_start(
                wt[:].rearrange("p (m k) -> p m k", m=15),
                wm.rearrange("m p k -> p m k"),
            )
            bt = const.tile([128, 2], dt.float32)
            nc.sync.dma_start(bt[:], bv[:])

            def W(i):
                return wt[:, i * 128:(i + 1) * 128]

            for rep in range(reps):
              for sg in range(NSG):
                pt = psip.tile([128, PTW], dt.bfloat16, tag="psi",
                               name=f"pt_{rep}_{sg}")
                nc.sync.dma_start(pt[:], psi[:, sg % nsg_io, :])
                ps = psip.tile([128, SW], dt.bfloat16, tag="psis",
                               name=f"ps_{rep}_{sg}")
                nc.gpsimd.dma_start(ps[:], psis[:, sg % nsg_io, :])

                # s1 = psi(x-1)+psi(x+1), s2 = psi(x-2)+psi(x+2) on DVE
                pt3 = pt[:].rearrange("p (qm x) -> p qm x", x=XP)
                s1t = spool.tile([128, SW], dt.bfloat16, tag="s1",
                                 name=f"s1_{rep}_{sg}")
                s2t = spool.tile([128, SW], dt.bfloat16, tag="s2",
                                 name=f"s2_{rep}_{sg}")
                s13 = s1t[:].rearrange("p (qm x) -> p qm x", x=RES)
                s23 = s2t[:].rearrange("p (qm x) -> p qm x", x=RES)
                nc.vector.tensor_add(s13, pt3[:, :, 1:1 + RES],
                                     pt3[:, :, 3:3 + RES])
                nc.vector.tensor_add(s23, pt3[:, :, 0:RES],
                                     pt3[:, :, 4:4 + RES])

                z1p = zpool.tile([128, ZW], dt.bfloat16, tag="z1p",
                                 name=f"z1p_{rep}_{sg}")

                # ---- stage 1: depthwise conv + lrelu (pair layout) ----
                for st in range(NST):
                    for h in range(2):       # q-pair halves {0,1},{2,3}
                        cps = psum.tile([128, 2 * S8 * RES], dt.float32,
                                        tag="p1", name=f"p1_{rep}_{sg}_{st}_{h}")
                        for _r in range(2 if probe == "pe2" else 1):
                          for qq in range(2):
                            q = 2 * h + qq
                            o3 = cps[:, qq * S8 * RES:(qq + 1) * S8 * RES]
                            # rhs slices: center from padded pt, s1, s2
                            rc = pt[:, q * SG * XP + st * S8 * XP:
                                    q * SG * XP + (st + 1) * S8 * XP]
                            rc3 = rc.rearrange("p (i x) -> p i x", x=XP)
                            r1 = s1t[:, q * SG * RES + st * S8 * RES:
                                     q * SG * RES + (st + 1) * S8 * RES]
                            r2 = s2t[:, q * SG * RES + st * S8 * RES:
                                     q * SG * RES + (st + 1) * S8 * RES]
                            o3r = o3.rearrange("p (i x) -> p i x", x=RES)
                            nc.tensor.matmul(o3r, W(0 * 4 + q),
                                             rc3[:, :, 2:2 + RES],
                                             start=True, stop=False)
                            nc.tensor.matmul(o3, W(1 * 4 + q), r1,
                                             start=False, stop=False)
                            nc.tensor.matmul(o3, W(2 * 4 + q), r2,
                                             start=False, stop=True)
                        # evacuate: lrelu -> z1p (bias1 == 0 by spec)
                        zv = z1p[:].rearrange("p (q mx) -> p q mx", q=4)
                        for _r in range(2 if probe == "act2" else 1):
                            nc.scalar.activation(
                                zv[:, 2 * h:2 * h + 2,
                                   st * S8 * RES:(st + 1) * S8 * RES],
                                cps[:].rearrange("p (q mx) -> p q mx", q=2),
                                LR, alpha=0.01)

                # ---- shuffle z1 pair -> shuf (8 SBUF->SBUF DMAs) ----
                z1s = zpool.tile([128, ZW], dt.bfloat16, tag="z1s",
                                 name=f"z1s_{rep}_{sg}")
                # pair->shuf via DRAM round-trip: 2 full-128-partition DMAs
                # (8 SBUF->SBUF DMAs concentrated writes on 16 partitions and
                # measured ~1.6us each; this spreads the fan evenly)
                zsl = zs[(rep * NSG + sg) % 4]
                nc.gpsimd.dma_start(
                    zsl.rearrange("q b j G m -> (b j G) q m"),
                    z1p[:].rearrange("p (q m) -> p q m", q=4))
                nc.sync.dma_start(
                    z1s[:].rearrange("p (G m) -> p G m", G=4),
                    zsl.rearrange("q b j G m -> (q b j) G m"))
                if probe == "dma2":
                    nc.sync.dma_start(
                        z1s[:].rearrange("p (G m) -> p G m", G=4),
                        zsl.rearrange("q b j G m -> (q b j) G m"))

                ot = opool.tile([128, OW], dt.bfloat16, tag="ot",
                                name=f"ot_{rep}_{sg}")

                # ---- stages 2+3 (shuf layout) ----
                for st in range(NST):
                    # stage 2: 8x8 GEMM + b2 + lrelu
                    z2 = zpool.tile([128, 4 * S8 * RES], dt.bfloat16,
                                    tag="z2", name=f"z2_{rep}_{sg}_{st}")
                    for h in range(2):       # G-pair halves
                        gps = psum.tile([128, 2 * S8 * RES], dt.float32,
                                        tag="p23", name=f"p2_{rep}_{sg}_{st}_{h}")
                        for gg in range(2):
                            G = 2 * h + gg
                            rz = z1s[:, G * SG * RES + st * S8 * RES:
                                     G * SG * RES + (st + 1) * S8 * RES]
                            nc.tensor.matmul(
                                gps[:, gg * S8 * RES:(gg + 1) * S8 * RES],
                                W(12), rz, start=True, stop=True)
                        nc.scalar.activation(
                            z2[:, 2 * h * S8 * RES:(2 * h + 2) * S8 * RES],
                            gps[:], LR, bias=bt[:, 0:1], alpha=0.01)
                    # stage 3: 8x8 GEMM + psi residual + b3 + tanh
                    for h in range(2):
                        gps = psum.tile([128, 2 * S8 * RES], dt.float32,
                                        tag="p23", name=f"p3_{rep}_{sg}_{st}_{h}")
                        for gg in range(2):
                            G = 2 * h + gg
                            o1 = gps[:, gg * S8 * RES:(gg + 1) * S8 * RES]
                            rp = ps[:, G * SG * RES + st * S8 * RES:
                                    G * SG * RES + (st + 1) * S8 * RES]
                            rz2 = z2[:, G * S8 * RES:(G + 1) * S8 * RES]
                            nc.tensor.matmul(o1, W(14), rp,
                                             start=True, stop=False)
                            nc.tensor.matmul(o1, W(13), rz2,
                                             start=False, stop=True)
                        ov = ot[:].rearrange("p (G mx) -> p G mx", G=4)
                        nc.scalar.activation(
                            ov[:, 2 * h:2 * h + 2,
                               st * S8 * RES:(st + 1) * S8 * RES],
                            gps[:].rearrange("p (G mx) -> p G mx", G=2),
                            TH, bias=bt[:, 1:2])

                nc.gpsimd.dma_start(out[:, sg % nsg_io, :], ot[:])

    nc.compile()
    return nc


def kernel(psi, filter1, bias1, w2, b2, w3, b3):
    from concourse.bass_utils import run_bass_kernel_spmd

    psi = np.asarray(psi, dtype=np.float32)
    wmv = _host_pack_weights(np.asarray(filter1, np.float32),
                             np.asarray(w2, np.float32),
                             np.asarray(w3, np.float32))
    bvv = _host_pack_biases(np.asarray(b2, np.float32),
                            np.asarray(b3, np.float32))
    psit = _pack_psi_pair(psi)
    psist = _pack_psi_shuf(psi)

    if "nc" not in _CACHE:
        _CACHE["nc"] = _build_program()
    nc = _CACHE["nc"]

    in_maps = [
        {"psi": psit[c], "psis": psist[c], "wm": wmv, "bv": bvv}
        for c in range(NCORES)
    ]
    res = run_bass_kernel_spmd(nc, in_maps, list(range(NCORES)))
    return _unpack_out([r["out"] for r in res.results])



# revision 2
# speedup vs baseline: 1.0186x; 1.0186x over previous
"""Trainium2 Bass kernel for the nn_CA depthwise-conv CA step (v4, bf16).

Pipeline per image: depthwise 5x5 conv (D4-symmetrized, zero-mean kernel,
SAME padding) + leaky_relu; 1x1 conv (8x8 channel GEMM) + bias + leaky_relu;
1x1 conv + bias + psi residual + tanh.

Strategy: pure data parallel over 8 NeuronCores (256 images each), all
tensors bf16 (rel err 7.6e-3 << 2e-2 tol), fp32 PSUM accumulate.

Two on-chip layouts per core:
 - "pair" (stage 1): partition p = b*64 + j*4 + G for channel c=2q+b at
   y = 16G + j; free = (q, image, padded x).  The y-conv is a (permuted)
   banded Toeplitz lhsT; D4 x-symmetry folds the 5 x-taps into 3 matmul
   terms: center column + s1 = psi(x-1)+psi(x+1) + s2 = psi(x-2)+psi(x+2),
   with s1/s2 computed on the DVE.
 - "shuf" (stages 2+3): partition p = c*16 + j (all 8 channels in the
   partition dim); free = (G, image, x).  The 8x8 channel GEMMs are ONE
   matmul per tile (full 8-channel contraction via w.T (x) I16 lhsT).
   The psi residual rides an accumulating I128 matmul from a host-packed
   shuf copy of psi.

The pair->shuf z1 reshuffle is 4 plain SBUF->SBUF DMAs (one per q:
z1s[32q:32q+32, :] <- z1p[:, 1024q:1024q+1024]; the flat iteration
orders coincide), issued from the gpsimd (SWDGE) ring so their
dependency waits never block the HBM streams, which all live on the
sync (HWDGE) ring.  This replaces the previous DRAM round-trip (saves
2.1 MB/supergroup of HBM traffic; measured 301 us -> 240 us with the
identical harness).  Biases + leaky_relu/tanh are fused into ScalarE
activations reading PSUM directly.  bias1 is always zero per the
problem spec and is not applied.
"""

import numpy as np
import ml_dtypes

BF16 = ml_dtypes.bfloat16

BS, H, RES = 2048, 8, 64
NCORES = 8
IPC = BS // NCORES   # images per core = 256
SG = 16              # images per supergroup (DMA granularity)
NSG = IPC // SG      # 16 supergroups
S8 = 8               # images per PSUM step
NST = SG // S8       # 2 steps per supergroup
XP = RES + 4         # x-padded width

# free-dim sizes
PTW = 4 * SG * XP        # psi pair tile width  = 4352
SW = 4 * SG * RES        # s1/s2 tile width     = 4096
ZW = 4 * SG * RES        # z1 pair/shuf width   = 4096
OW = 4 * SG * RES        # out tile width       = 4096

_CACHE = {}

# partition interleave: y = 16*G + j  ->  pi(y) = j*4 + G
_PI = np.array([(y % 16) * 4 + y // 16 for y in range(64)])


def _totalistic(x):
    z = 0.125 * (x + x[:, :, ::-1, :] + x[:, :, :, ::-1] + x[:, :, ::-1, ::-1])
    xt = np.swapaxes(x, 2, 3)
    z = z + 0.125 * (xt + xt[:, :, ::-1, :] + xt[:, :, :, ::-1] + xt[:, :, ::-1, ::-1])
    return z - z.mean(axis=(2, 3), keepdims=True)


def _host_pack_weights(filter1, w2, w3):
    """-> wm [15, 128, 128] bf16: 12 stage-1 permuted-Toeplitz lhsTs
    (t*4+q), W2 shuf lhsT (12), W3 shuf lhsT (13), I128 (14)."""
    K = _totalistic(filter1.astype(np.float32))[:, 0]  # [8,5,5]
    wm = np.zeros((15, 128, 128), np.float32)
    # stage 1: z[y'] = sum_dy K[dy+2, col_t] * s_t[y'+dy]
    for t, col in enumerate((2, 1, 0)):
        for q in range(4):
            m = wm[t * 4 + q]
            for b in range(2):
                k = K[2 * q + b, :, col]
                for yp in range(64):
                    for dy in range(-2, 3):
                        y = yp + dy
                        if 0 <= y < 64:
                            m[b * 64 + _PI[y], b * 64 + _PI[yp]] = k[dy + 2]
    wm[12] = np.kron(w2.T.astype(np.float32), np.eye(16, dtype=np.float32))
    wm[13] = np.kron(w3.T.astype(np.float32), np.eye(16, dtype=np.float32))
    wm[14] = np.eye(128, dtype=np.float32)
    return wm.astype(BF16)


def _host_pack_biases(b2, b3):
    bv = np.zeros((128, 2), np.float32)
    p = np.arange(128)
    bv[:, 0] = b2[p // 16]
    bv[:, 1] = b3[p // 16]
    return bv


def _pack_psi_pair(psi):
    """[BS,H,RES,RES] -> [NC, 128, NSG, PTW] bf16, partition b*64+j*4+G,
    free (q, m, x+2) with zero x-pad baked in."""
    # (core, sg, m, q, b, G, j, x)
    v = psi.reshape(NCORES, NSG, SG, 4, 2, 4, 16, RES)
    v = v.transpose(0, 4, 6, 5, 1, 3, 2, 7)  # core, b, j, G, sg, q, m, x
    vp = np.zeros((NCORES, 2, 16, 4, NSG, 4, SG, XP), np.float32)
    vp[..., 2:2 + RES] = v
    return np.ascontiguousarray(vp).astype(BF16).reshape(NCORES, 128, NSG, PTW)


def _pack_psi_shuf(psi):
    """[BS,H,RES,RES] -> [NC, 128, NSG, SW] bf16, partition c*16+j,
    free (G, m, x)."""
    # (core, sg, m, c, G, j, x)
    v = psi.reshape(NCORES, NSG, SG, H, 4, 16, RES)
    v = v.transpose(0, 3, 5, 1, 4, 2, 6)  # core, c, j, sg, G, m, x
    return np.ascontiguousarray(v).astype(BF16).reshape(NCORES, 128, NSG, SW)


def _unpack_out(parts):
    """list of [128, NSG, OW] bf16 per core -> [BS,H,RES,RES] fp32"""
    v = np.stack(parts).astype(np.float32)  # [NC, 128, NSG, OW]
    v = v.reshape(NCORES, H, 16, NSG, 4, SG, RES)  # core, c, j, sg, G, m, x
    v = v.transpose(0, 3, 5, 1, 4, 2, 6)  # core, sg, m, c, G, j, x
    return np.ascontiguousarray(v).reshape(BS, H, RES, RES)


def _build_program(reps=1, nsg_io=NSG, probe=None, dma_map=None):
    """nsg_io < NSG builds a timing variant: identical instruction stream,
    but DRAM I/O cycles through a small nsg_io-supergroup buffer so the
    host<->device shipping (which axon does per exec) is negligible.

    dma_map: which engine issues each DMA flow.  Keys: 'psi', 'psis',
    'out', 'shuf' (4-list).  Values: 'sync'|'scalar'|'gpsimd'."""
    import concourse.bacc as bacc
    import concourse.tile as tile
    from concourse import mybir

    if dma_map is None:
        dma_map = {"psi": "sync", "psis": "sync", "out": "sync",
                   "shuf": ["gpsimd"] * 4}

    dt = mybir.dt
    nc = bacc.Bacc("TRN2", target_bir_lowering=False, debug=False,
                   num_devices=NCORES)
    psi = nc.dram_tensor("psi", [128, nsg_io, PTW], dt.bfloat16,
                         kind="ExternalInput").ap()
    psis = nc.dram_tensor("psis", [128, nsg_io, SW], dt.bfloat16,
                          kind="ExternalInput").ap()
    wm = nc.dram_tensor("wm", [15, 128, 128], dt.bfloat16,
                        kind="ExternalInput").ap()
    bv = nc.dram_tensor("bv", [128, 2], dt.float32,
                        kind="ExternalInput").ap()
    out = nc.dram_tensor("out", [128, nsg_io, OW], dt.bfloat16,
                         kind="ExternalOutput").ap()

    LR = mybir.ActivationFunctionType.Lrelu
    TH = mybir.ActivationFunctionType.Tanh

    with tile.TileContext(nc) as tc:
        from contextlib import ExitStack

        def ENG(name):
            return {"sync": nc.sync, "scalar": nc.scalar,
                    "gpsimd": nc.gpsimd}[name]

        with ExitStack() as ctx:
            const = ctx.enter_context(tc.tile_pool(name="const", bufs=1))
            psip = ctx.enter_context(tc.tile_pool(name="psip", bufs=3))
            spool = ctx.enter_context(tc.tile_pool(name="spool", bufs=2))
            zpool = ctx.enter_context(tc.tile_pool(name="zpool", bufs=3))
            opool = ctx.enter_context(tc.tile_pool(name="opool", bufs=3))
            lpool = ctx.enter_context(tc.tile_pool(name="lpool", bufs=2))
            merge_h = dma_map.get("merge_h", False)
            # merged-h psum tiles are 4 banks each (p1 + p23 = 8 = all of
            # PSUM), so single-buffered; unmerged 2-bank tiles double-buffer.
            psum = ctx.enter_context(tc.tile_pool(name="psum",
                                                  bufs=1 if merge_h else 2,
                                                  space="PSUM"))

            wt = const.tile([128, 15 * 128], dt.bfloat16)
            nc.sync.dma_start(
                wt[:].rearrange("p (m k) -> p m k", m=15),
                wm.rearrange("m p k -> p m k"),
            )
            bt = const.tile([128, 2], dt.float32)
            nc.sync.dma_start(bt[:], bv[:])

            def W(i):
                return wt[:, i * 128:(i + 1) * 128]

            # Software-pipelined emission: front half (loads + x-shift
            # adds) for iteration i is emitted BEFORE the back half
            # (conv/lrelu/shuffle/GEMMs/store) of iteration i-1, so the
            # scheduler naturally orders DVE s-adds(i) ahead of any
            # DVE-offloaded lrelu(i-1) and keeps every engine fed.
            swpipe = dma_map.get("swpipe", True)
            total = reps * NSG
            carried = {}

            def front(i):
                sg = i % NSG
                pt = psip.tile([128, PTW], dt.bfloat16, tag="psi",
                               name=f"pt_{i}")
                ENG(dma_map["psi"]).dma_start(pt[:], psi[:, sg % nsg_io, :])
                ps = psip.tile([128, SW], dt.bfloat16, tag="psis",
                               name=f"ps_{i}")
                ENG(dma_map["psis"]).dma_start(ps[:], psis[:, sg % nsg_io, :])

                # s1 = psi(x-1)+psi(x+1), s2 = psi(x-2)+psi(x+2) on DVE
                pt3 = pt[:].rearrange("p (qm x) -> p qm x", x=XP)
                s1t = spool.tile([128, SW], dt.bfloat16, tag="s1",
                                 name=f"s1_{i}")
                s2t = spool.tile([128, SW], dt.bfloat16, tag="s2",
                                 name=f"s2_{i}")
                s13 = s1t[:].rearrange("p (qm x) -> p qm x", x=RES)
                s23 = s2t[:].rearrange("p (qm x) -> p qm x", x=RES)
                nc.vector.tensor_add(s13, pt3[:, :, 1:1 + RES],
                                     pt3[:, :, 3:3 + RES])
                nc.vector.tensor_add(s23, pt3[:, :, 0:RES],
                                     pt3[:, :, 4:4 + RES])
                carried[i] = (pt, ps, s1t, s2t)

            for i in range(total + (1 if swpipe else 0)):
                if i < total:
                    front(i)
                if swpipe and i == 0:
                    continue
                j = (i - 1) if swpipe else i
                pt, ps, s1t, s2t = carried.pop(j)
                sg = j % NSG
                rep = j // NSG

                z1p = zpool.tile([128, ZW], dt.bfloat16, tag="z1p",
                                 name=f"z1p_{rep}_{sg}")

                # ---- stage 1: depthwise conv + lrelu (pair layout) ----
                # merge_h: one 4-bank psum tile per st covering all 4 q
                # planes, evacuated by a single FD-2048 activation (halves
                # the ACT instruction count; ACT is the pacing engine).
                for st in range(NST):
                  if merge_h:
                    cps = psum.tile([128, 4 * S8 * RES], dt.float32,
                                    tag="p1", name=f"p1_{rep}_{sg}_{st}")
                    for q in range(4):
                        o3 = cps[:, q * S8 * RES:(q + 1) * S8 * RES]
                        rc = pt[:, q * SG * XP + st * S8 * XP:
                                q * SG * XP + (st + 1) * S8 * XP]
                        rc3 = rc.rearrange("p (i x) -> p i x", x=XP)
                        r1 = s1t[:, q * SG * RES + st * S8 * RES:
                                 q * SG * RES + (st + 1) * S8 * RES]
                        r2 = s2t[:, q * SG * RES + st * S8 * RES:
                                 q * SG * RES + (st + 1) * S8 * RES]
                        o3r = o3.rearrange("p (i x) -> p i x", x=RES)
                        nc.tensor.matmul(o3r, W(0 * 4 + q),
                                         rc3[:, :, 2:2 + RES],
                                         start=True, stop=False)
                        nc.tensor.matmul(o3, W(1 * 4 + q), r1,
                                         start=False, stop=False)
                        nc.tensor.matmul(o3, W(2 * 4 + q), r2,
                                         start=False, stop=True)
                    zv = z1p[:].rearrange("p (q mx) -> p q mx", q=4)
                    if dma_map.get("dve_lrelu_st1"):
                        lt = lpool.tile([128, 4 * S8 * RES], dt.float32,
                                        tag="lr", name=f"lr_{rep}_{sg}_{st}")
                        nc.vector.tensor_scalar_mul(lt[:], cps[:], 0.01)
                        nc.vector.tensor_max(
                            zv[:, :, st * S8 * RES:(st + 1) * S8 * RES],
                            cps[:].rearrange("p (q mx) -> p q mx", q=4),
                            lt[:].rearrange("p (q mx) -> p q mx", q=4))
                    else:
                        nc.scalar.activation(
                            zv[:, :, st * S8 * RES:(st + 1) * S8 * RES],
                            cps[:].rearrange("p (q mx) -> p q mx", q=4),
                            LR, alpha=0.01)
                  else:
                    for h in range(2):       # q-pair halves {0,1},{2,3}
                        cps = psum.tile([128, 2 * S8 * RES], dt.float32,
                                        tag="p1", name=f"p1_{rep}_{sg}_{st}_{h}")
                        for qq in range(2):
                            q = 2 * h + qq
                            o3 = cps[:, qq * S8 * RES:(qq + 1) * S8 * RES]
                            # rhs slices: center from padded pt, s1, s2
                            rc = pt[:, q * SG * XP + st * S8 * XP:
                                    q * SG * XP + (st + 1) * S8 * XP]
                            rc3 = rc.rearrange("p (i x) -> p i x", x=XP)
                            r1 = s1t[:, q * SG * RES + st * S8 * RES:
                                     q * SG * RES + (st + 1) * S8 * RES]
                            r2 = s2t[:, q * SG * RES + st * S8 * RES:
                                     q * SG * RES + (st + 1) * S8 * RES]
                            o3r = o3.rearrange("p (i x) -> p i x", x=RES)
                            nc.tensor.matmul(o3r, W(0 * 4 + q),
                                             rc3[:, :, 2:2 + RES],
                                             start=True, stop=False)
                            nc.tensor.matmul(o3, W(1 * 4 + q), r1,
                                             start=False, stop=False)
                            nc.tensor.matmul(o3, W(2 * 4 + q), r2,
                                             start=False, stop=True)
                        zv = z1p[:].rearrange("p (q mx) -> p q mx", q=4)
                        zslice = zv[:, 2 * h:2 * h + 2,
                                    st * S8 * RES:(st + 1) * S8 * RES]
                        if h in dma_map.get("dve_lrelu_h", ()):
                            lt = lpool.tile([128, 2 * S8 * RES], dt.float32,
                                            tag="lr", name=f"lr_{rep}_{sg}_{st}_{h}")
                            nc.vector.tensor_scalar_mul(lt[:], cps[:], 0.01)
                            nc.vector.tensor_max(
                                zslice,
                                cps[:].rearrange("p (q mx) -> p q mx", q=2),
                                lt[:].rearrange("p (q mx) -> p q mx", q=2))
                        else:
                            nc.scalar.activation(
                                zslice,
                                cps[:].rearrange("p (q mx) -> p q mx", q=2),
                                LR, alpha=0.01)

                # ---- shuffle z1 pair -> shuf ----
                z1s = zpool.tile([128, ZW], dt.bfloat16, tag="z1s",
                                 name=f"z1s_{rep}_{sg}")
                # pair->shuf via 4 plain SBUF->SBUF DMAs (one per q).  The
                # flat iteration orders line up exactly: out partitions
                # 32q+16b+j (=(2q+b)*16+j) x free (G,m) <- in partitions
                # (b,j,G) x free m at offset 1024q.  Avoids the DRAM
                # round-trip's 2x HBM traffic.  q pairs go on the two
                # HWDGE rings (sync/scalar); each q's 32 write partitions
                # hit a disjoint half of the SDMA engines (even/odd), so
                # the two rings don't contend.
                for q in range(4):
                    ENG(dma_map["shuf"][q]).dma_start(
                        z1s[32 * q:32 * (q + 1), :],
                        z1p[:, SG * RES * q:SG * RES * (q + 1)])

                ot = opool.tile([128, OW], dt.bfloat16, tag="ot",
                                name=f"ot_{rep}_{sg}")

                # ---- stages 2+3 (shuf layout) ----
                for st in range(NST):
                    # stage 2: 8x8 GEMM + b2 + lrelu
                    z2 = zpool.tile([128, 4 * S8 * RES], dt.bfloat16,
                                    tag="z2", name=f"z2_{rep}_{sg}_{st}")
                    if merge_h:
                        gps = psum.tile([128, 4 * S8 * RES], dt.float32,
                                        tag="p23", name=f"p2_{rep}_{sg}_{st}")
                        for G in range(4):
                            rz = z1s[:, G * SG * RES + st * S8 * RES:
                                     G * SG * RES + (st + 1) * S8 * RES]
                            nc.tensor.matmul(
                                gps[:, G * S8 * RES:(G + 1) * S8 * RES],
                                W(12), rz, start=True, stop=True)
                        nc.scalar.activation(z2[:], gps[:], LR,
                                             bias=bt[:, 0:1], alpha=0.01)
                    else:
                      for h in range(2):       # G-pair halves
                        gps = psum.tile([128, 2 * S8 * RES], dt.float32,
                                        tag="p23", name=f"p2_{rep}_{sg}_{st}_{h}")
                        for gg in range(2):
                            G = 2 * h + gg
                            rz = z1s[:, G * SG * RES + st * S8 * RES:
                                     G * SG * RES + (st + 1) * S8 * RES]
                            nc.tensor.matmul(
                                gps[:, gg * S8 * RES:(gg + 1) * S8 * RES],
                                W(12), rz, start=True, stop=True)
                        nc.scalar.activation(
                            z2[:, 2 * h * S8 * RES:(2 * h + 2) * S8 * RES],
                            gps[:], LR, bias=bt[:, 0:1], alpha=0.01)
                    # stage 3: 8x8 GEMM + psi residual + b3 + tanh
                    ov = ot[:].rearrange("p (G mx) -> p G mx", G=4)
                    if merge_h:
                        gps = psum.tile([128, 4 * S8 * RES], dt.float32,
                                        tag="p23", name=f"p3_{rep}_{sg}_{st}")
                        for G in range(4):
                            o1 = gps[:, G * S8 * RES:(G + 1) * S8 * RES]
                            rp = ps[:, G * SG * RES + st * S8 * RES:
                                    G * SG * RES + (st + 1) * S8 * RES]
                            rz2 = z2[:, G * S8 * RES:(G + 1) * S8 * RES]
                            nc.tensor.matmul(o1, W(14), rp,
                                             start=True, stop=False)
                            nc.tensor.matmul(o1, W(13), rz2,
                                             start=False, stop=True)
                        nc.scalar.activation(
                            ov[:, :, st * S8 * RES:(st + 1) * S8 * RES],
                            gps[:].rearrange("p (G mx) -> p G mx", G=4),
                            TH, bias=bt[:, 1:2])
                    else:
                      for h in range(2):
                        gps = psum.tile([128, 2 * S8 * RES], dt.float32,
                                        tag="p23", name=f"p3_{rep}_{sg}_{st}_{h}")
                        for gg in range(2):
                            G = 2 * h + gg
                            o1 = gps[:, gg * S8 * RES:(gg + 1) * S8 * RES]
                            rp = ps[:, G * SG * RES + st * S8 * RES:
                                    G * SG * RES + (st + 1) * S8 * RES]
                            rz2 = z2[:, G * S8 * RES:(G + 1) * S8 * RES]
                            nc.tensor.matmul(o1, W(14), rp,
                                             start=True, stop=False)
                            nc.tensor.matmul(o1, W(13), rz2,
                                             start=False, stop=True)
                        nc.scalar.activation(
                            ov[:, 2 * h:2 * h + 2,
                               st * S8 * RES:(st + 1) * S8 * RES],
                            gps[:].rearrange("p (G mx) -> p G mx", G=2),
                            TH, bias=bt[:, 1:2])

                ENG(dma_map["out"]).dma_start(out[:, sg % nsg_io, :], ot[:])

    nc.compile()
    return nc


def kernel(psi, filter1, bias1, w2, b2, w3, b3):
    from concourse.bass_utils import run_bass_kernel_spmd

    psi = np.asarray(psi, dtype=np.float32)
    wmv = _host_pack_weights(np.asarray(filter1, np.float32),
                             np.asarray(w2, np.float32),
                             np.asarray(w3, np.float32))
    bvv = _host_pack_biases(np.asarray(b2, np.float32),
                            np.asarray(b3, np.float32))
    psit = _pack_psi_pair(psi)
    psist = _pack_psi_shuf(psi)

    if "nc" not in _CACHE:
        _CACHE["nc"] = _build_program()
    nc = _CACHE["nc"]

    in_maps = [
        {"psi": psit[c], "psis": psist[c], "wm": wmv, "bv": bvv}
        for c in range(NCORES)
    ]
    res = run_bass_kernel_spmd(nc, in_maps, list(range(NCORES)))
    return _unpack_out([r["out"] for r in res.results])

